# revision 1
# baseline (speedup 1.0000x reference)
"""2-layer GAT on 8 Trainium2 NeuronCores.

Strategy
--------
Core c owns destination nodes [c*12500, (c+1)*12500); every edge lives on the
core that owns its destination, so the scatter-softmax segment reduction is
entirely core-local. Between layers, only a small AllGather of per-node
feature tables ([H | alpha_src | 1] rows) crosses cores.

Per core, destination nodes are bucketed by local in-degree padded to a
multiple of 8 (R). Each node owns exactly R contiguous "slot" columns in a
[128, T_U] slot grid; real edges fill the first deg slots, the rest point at
an all-zero dummy table row. This makes the segment softmax/reduction a set
of REGULAR strided DVE ops (no scatter at all). The only irreducible random
access is the per-slot gather of table rows Haug[src], done with one
indirect DMA per chunk (per-index descriptors).

Softmax is computed without the max-subtraction: attention logits here are
bounded (|e| < ~10) so exp() is safe in fp32, and the result is identical to
the reference's stabilized form up to rounding. Normalization happens once
per node after the weighted sum: out = U[:, :D] / (U[:, D+1] + 1e-16).

The Bass program is built AFTER seeing the inputs (shapes/layouts baked in),
compiled via neuronx-cc, and run SPMD on 8 cores through the PJRT path.
"""
import sys

sys.path.insert(0, "/opt/trn_rl_repo")

import numpy as np

P = 128
N_NODES = 100000
N_CORES = 8
IN_DIM = 256
HID = 8
OUT = 16
NEG = 0.2


class _Meta:
    pass


def _preprocess(E, X, RQ=4, target_chunk=512):
    N, C = N_NODES, N_CORES
    NLOC = N // C
    src = np.asarray(E[0], dtype=np.int64)
    dst = np.asarray(E[1], dtype=np.int64)

    deg = np.zeros((C, NLOC), dtype=np.int64)
    np.add.at(deg.reshape(-1), dst, 1)


    # Data-adaptive bucket boundaries (DP): minimize total slot columns
    # sum_b nrow_b * R_b over degree-range buckets, instead of fixed RQ.
    dmax = int(deg.max())
    cntd = np.zeros((C, dmax + 1), dtype=np.int64)
    for c in range(C):
        cntd[c] = np.bincount(deg[c][deg[c] > 0], minlength=dmax + 1)
    pred = cntd.cumsum(axis=1)
    INF = 1 << 60
    fdp = [0] + [INF] * dmax
    chx = [0] * (dmax + 1)
    for j in range(1, dmax + 1):
        for i in range(1, j + 1):
            n = pred[:, j] - pred[:, i - 1]
            v = fdp[i - 1] + int(np.ceil(n.max() / P)) * j
            if v < fdp[j]:
                fdp[j] = v
                chx[j] = i
    deg2R = np.zeros(dmax + 1, dtype=np.int64)
    j = dmax
    while j > 0:
        i = chx[j]
        deg2R[i:j + 1] = j
        j = i - 1
    Rv = deg2R[deg]

    Rs = sorted(set(int(r) for r in np.unique(Rv) if r > 0))
    has_zero = bool((Rv == 0).any())
    Rs_cells = Rs + ([0] if has_zero else [])

    nrow = {}
    for R in Rs_cells:
        cnt = (Rv == R).sum(axis=1)
        nrow[R] = int(np.ceil(cnt.max() / P))
    nrow_tot = sum(nrow.values()) + 1
    NR = P * nrow_tot
    DUMMY = C * NR - 1

    colbase = {}
    cb = 0
    for R in Rs_cells:
        colbase[R] = cb
        cb += nrow[R]
    slotbase = {}
    sb = 0
    for R in Rs:
        slotbase[R] = sb
        sb += nrow[R] * R
    T_U = sb

    meta = _Meta()
    meta.N, meta.C, meta.NLOC, meta.NR = N, C, NLOC, NR
    meta.nrow_tot, meta.T_U, meta.Rs = nrow_tot, T_U, Rs
    meta.nrow, meta.colbase, meta.slotbase = nrow, colbase, slotbase
    meta.DUMMY = DUMMY

    cell2node = np.full((C, P, nrow_tot), -1, dtype=np.int64)
    tabrow_of = np.full(N, DUMMY, dtype=np.int64)
    for c in range(C):
        for R in Rs_cells:
            ls = np.nonzero(Rv[c] == R)[0]
            k = np.arange(len(ls))
            p = k % P
            i = colbase[R] + k // P
            cell2node[c, p, i] = ls
            tabrow_of[c * NLOC + ls] = c * NR + p * nrow_tot + i
    meta.cell2node = cell2node
    meta.ones = (cell2node >= 0).astype(np.float32)

    idx = np.full((C, P, T_U), DUMMY, dtype=np.int64)
    order = np.argsort(dst, kind="stable")
    s_src = src[order]
    s_dst = dst[order]
    grp_start = np.searchsorted(s_dst, np.arange(N))
    pos_in_grp = np.arange(len(s_dst)) - grp_start[s_dst]
    e_c = s_dst // NLOC
    e_l = s_dst % NLOC
    e_R = Rv[e_c, e_l]
    e_k = np.zeros(len(s_dst), dtype=np.int64)
    for c in range(C):
        for R in Rs:
            ls = np.nonzero(Rv[c] == R)[0]
            rank = np.full(NLOC, -1, dtype=np.int64)
            rank[ls] = np.arange(len(ls))
            m = (e_c == c) & (e_R == R)
            e_k[m] = rank[e_l[m]]
    e_p = e_k % P
    e_i_off = e_k // P
    sb_arr = np.array([slotbase[int(r)] for r in e_R])
    e_t = sb_arr + e_i_off * e_R + pos_in_grp
    idx[e_c, e_p, e_t] = tabrow_of[s_src]
    meta.idx = idx.astype(np.int32)

    XcT = np.zeros((C, IN_DIM, NR), dtype=np.float32)
    Xf = np.asarray(X, np.float32)
    for c in range(C):
        pp, ii = np.nonzero(cell2node[c] >= 0)
        ls = cell2node[c, pp, ii]
        XcT[c][:, ii * P + pp] = Xf[c * NLOC + ls].T
    meta.XcT = XcT

    col_R = np.zeros(nrow_tot, dtype=np.int64)
    col_sb = np.zeros(nrow_tot + 1, dtype=np.int64)
    for R in Rs:
        col_R[colbase[R]:colbase[R] + nrow[R]] = R
    acc = 0
    for i in range(nrow_tot):
        col_sb[i] = acc
        acc += col_R[i]
    col_sb[nrow_tot] = acc
    chunks = []
    i0 = 0
    while i0 < nrow_tot and col_R[i0] > 0:
        i1 = i0
        while (i1 < nrow_tot and col_R[i1] > 0
               and col_sb[i1 + 1] - col_sb[i0] <= target_chunk):
            i1 += 1
        inters = []
        for R in Rs:
            ia = max(i0, colbase[R])
            ib = min(i1, colbase[R] + nrow[R])
            if ia < ib:
                inters.append((R, ia, ib, int(col_sb[ia])))
        chunks.append((i0, i1, int(col_sb[i0]), int(col_sb[i1]), inters))
        i0 = i1
    meta.chunks = chunks
    return meta


def _build(meta):
    import concourse.bass as bass
    import concourse.bacc as bacc
    import concourse.mybir as mybir
    import concourse.tile as tile

    F32 = mybir.dt.float32
    I32 = mybir.dt.int32
    AX = mybir.AxisListType
    OP = mybir.AluOpType
    AF = mybir.ActivationFunctionType

    C, NR, nt, T_U = meta.C, meta.NR, meta.nrow_tot, meta.T_U
    D1, D2 = HID + 2, OUT + 2
    KCH = IN_DIM // P

    nc = bacc.Bacc()
    XcT_d = nc.declare_dram_parameter("XcT", [IN_DIM, NR], F32, isOutput=False)
    ones_d = nc.declare_dram_parameter("ones", [P, nt], F32, isOutput=False)
    idx_d = nc.declare_dram_parameter("idx", [P, T_U], I32, isOutput=False)
    W1_d = nc.declare_dram_parameter("W1", [IN_DIM, HID], F32, isOutput=False)
    a1s_d = nc.declare_dram_parameter("a1s", [1, HID], F32, isOutput=False)
    a1d_d = nc.declare_dram_parameter("a1d", [1, HID], F32, isOutput=False)
    W2T_d = nc.declare_dram_parameter("W2T", [1, OUT * HID], F32, isOutput=False)
    a2s_d = nc.declare_dram_parameter("a2s", [1, OUT], F32, isOutput=False)
    a2d_d = nc.declare_dram_parameter("a2d", [1, OUT], F32, isOutput=False)
    out_d = nc.declare_dram_parameter("out", [P, nt * OUT], F32, isOutput=True)

    cc1_d = nc.dram_tensor("cc1", [P, nt * D1], F32)
    tab1_d = nc.dram_tensor("tab1", [C * NR, D1], F32, addr_space="Shared")
    cc2_d = nc.dram_tensor("cc2", [P, nt * D2], F32)
    tab2_d = nc.dram_tensor("tab2", [C * NR, D2], F32, addr_space="Shared")
    groups = [list(range(C))]

    with tile.TileContext(nc) as tc:
        with (
            tc.tile_pool(name="persist", bufs=1) as pp,
            tc.tile_pool(name="xs", bufs=2) as xp,
            tc.tile_pool(name="gp", bufs=2) as gp,
            tc.tile_pool(name="ew", bufs=2) as ewp,
            tc.tile_pool(name="tmp", bufs=1) as tp,
            tc.tile_pool(name="ps", bufs=8, space="PSUM") as psp,
        ):
            idx_t = pp.tile([P, T_U], I32, tag="idx")
            nc.sync.dma_start(out=idx_t[:], in_=idx_d[:])
            ones_t = pp.tile([P, nt], F32, tag="ones")
            nc.sync.dma_start(out=ones_t[:], in_=ones_d[:])
            w1_t = pp.tile([P, KCH * HID], F32, tag="w1")
            for k in range(KCH):
                nc.sync.dma_start(out=w1_t[:, k * HID:(k + 1) * HID],
                                  in_=W1_d[k * P:(k + 1) * P, :])
            a1s_t = pp.tile([P, HID], F32, tag="a1s")
            nc.sync.dma_start(out=a1s_t[:], in_=a1s_d[0:1, :].to_broadcast([P, HID]))
            a1d_t = pp.tile([P, HID], F32, tag="a1d")
            nc.sync.dma_start(out=a1d_t[:], in_=a1d_d[0:1, :].to_broadcast([P, HID]))
            w2t_t = pp.tile([P, OUT * HID], F32, tag="w2t")
            nc.sync.dma_start(out=w2t_t[:],
                              in_=W2T_d[0:1, :].to_broadcast([P, OUT * HID]))
            a2s_t = pp.tile([P, OUT], F32, tag="a2s")
            nc.sync.dma_start(out=a2s_t[:], in_=a2s_d[0:1, :].to_broadcast([P, OUT]))
            a2d_t = pp.tile([P, OUT], F32, tag="a2d")
            nc.sync.dma_start(out=a2d_t[:], in_=a2d_d[0:1, :].to_broadcast([P, OUT]))

            # layer-1 node table: H = X @ W1 per 128-node block
            hg = pp.tile([P, nt * D1], F32, tag="hg")
            XB = 8
            for b0 in range(0, nt, XB):
                b1 = min(b0 + XB, nt)
                nb = b1 - b0
                xt = xp.tile([P, KCH * XB * P], F32, tag="xt")
                for k in range(KCH):
                    nc.sync.dma_start(out=xt[:, k * XB * P:k * XB * P + nb * P],
                                      in_=XcT_d[k * P:(k + 1) * P, b0 * P:b1 * P])
                for b in range(b0, b1):
                    ps = psp.tile([P, HID], F32, tag="hps")
                    for k in range(KCH):
                        nc.tensor.matmul(
                            out=ps[:],
                            lhsT=xt[:, k * XB * P + (b - b0) * P:
                                    k * XB * P + (b - b0 + 1) * P],
                            rhs=w1_t[:, k * HID:(k + 1) * HID],
                            start=(k == 0), stop=(k == KCH - 1))
                    nc.scalar.copy(out=hg[:, b * D1:b * D1 + HID], in_=ps[:])
            hv = hg[:].rearrange("p (n j) -> p n j", j=D1)[:, :, 0:HID]
            t_a = tp.tile([P, nt * HID], F32, tag="amul")
            tv = t_a[:].rearrange("p (n j) -> p n j", j=HID)
            nc.vector.tensor_tensor(out=tv, in0=hv,
                                    in1=a1s_t[:, None, :].to_broadcast([P, nt, HID]),
                                    op=OP.mult)
            nc.vector.tensor_reduce(
                out=hg[:].rearrange("p (n j) -> p n j", j=D1)[:, :, HID:HID + 1],
                in_=tv[:, :, None, :], axis=AX.X, op=OP.add)
            ad1_t = pp.tile([P, nt], F32, tag="ad1")
            t_b = tp.tile([P, nt * HID], F32, tag="amul")
            tv2 = t_b[:].rearrange("p (n j) -> p n j", j=HID)
            nc.vector.tensor_tensor(out=tv2, in0=hv,
                                    in1=a1d_t[:, None, :].to_broadcast([P, nt, HID]),
                                    op=OP.mult)
            nc.vector.tensor_reduce(out=ad1_t[:, :, None], in_=tv2[:, :, None, :],
                                    axis=AX.X, op=OP.add)
            nc.vector.tensor_copy(
                out=hg[:].rearrange("p (n j) -> p n j", j=D1)[:, :, HID + 1:HID + 2],
                in_=ones_t[:, :, None])
            nc.sync.dma_start(out=cc1_d[:], in_=hg[:])
            nc.gpsimd.collective_compute(
                "AllGather", OP.bypass, replica_groups=groups,
                ins=[cc1_d[:]], outs=[tab1_d[:]])

            def edge_layer(tab_d, D, ad_t, U):
                for (i0, i1, s0, s1, inters) in meta.chunks:
                    SC = s1 - s0
                    g_t = gp.tile([P, SC * D2], F32, tag="g")
                    gD = g_t[:, :SC * D]
                    # HW indirect DMA supports exactly one offset per
                    # partition with a 2D dest: one instruction per slot col.
                    for t in range(s0, s1):
                        nc.gpsimd.indirect_dma_start(
                            out=g_t[:, (t - s0) * D:(t - s0 + 1) * D],
                            out_offset=None, in_=tab_d[:],
                            in_offset=bass.IndirectOffsetOnAxis(
                                ap=idx_t[:, t:t + 1], axis=0))
                    e_t = ewp.tile([P, SC], F32, tag="e")
                    w_t = ewp.tile([P, SC], F32, tag="w")
                    for (R, ia, ib, sa) in inters:
                        nn = ib - ia
                        o = sa - s0
                        ev = e_t[:, o:o + nn * R].rearrange("p (n r) -> p n r", r=R)
                        gv = gD[:, o * D:(o + nn * R) * D].rearrange(
                            "p (n r j) -> p n r j", r=R, j=D)[:, :, :, D - 2]
                        adv = ad_t[:, ia:ib, None].to_broadcast([P, nn, R])
                        nc.vector.tensor_tensor(out=ev, in0=gv, in1=adv, op=OP.add)
                    nc.vector.tensor_scalar_mul(w_t[:], e_t[:], NEG)
                    nc.vector.tensor_tensor(out=w_t[:], in0=w_t[:], in1=e_t[:],
                                            op=OP.max)
                    nc.scalar.activation(w_t[:], w_t[:], AF.Exp)
                    nc.vector.tensor_tensor(
                        out=gD.rearrange("p (s j) -> p s j", j=D),
                        in0=gD.rearrange("p (s j) -> p s j", j=D),
                        in1=w_t[:, :, None].to_broadcast([P, SC, D]), op=OP.mult)
                    for (R, ia, ib, sa) in inters:
                        nn = ib - ia
                        o = sa - s0
                        uv = U[:, ia * D:ib * D].rearrange("p (n j) -> p n j", j=D)
                        gv = gD[:, o * D:(o + nn * R) * D].rearrange(
                            "p (n r j) -> p n j r", r=R, j=D)
                        nc.vector.tensor_reduce(out=uv, in_=gv, axis=AX.X, op=OP.add)

            U1 = pp.tile([P, nt * D1], F32, tag="U1")
            nc.vector.memset(U1[:], 0.0)
            edge_layer(tab1_d, D1, ad1_t, U1)

            z_t = tp.tile([P, nt], F32, tag="z")
            nc.vector.tensor_scalar_add(
                z_t[:, :, None],
                U1[:].rearrange("p (n j) -> p n j", j=D1)[:, :, D1 - 1:D1], 1e-16)
            rec_t = tp.tile([P, nt], F32, tag="rec")
            nc.vector.reciprocal(rec_t[:], z_t[:])
            h2 = pp.tile([P, nt * HID], F32, tag="h2")
            h2v = h2[:].rearrange("p (n j) -> p n j", j=HID)
            nc.vector.tensor_tensor(
                out=h2v, in0=U1[:].rearrange("p (n j) -> p n j", j=D1)[:, :, 0:HID],
                in1=rec_t[:, :, None].to_broadcast([P, nt, HID]), op=OP.mult)
            tneg = tp.tile([P, nt * HID], F32, tag="telu")
            nc.vector.tensor_scalar_min(tneg[:], h2[:], 0.0)
            nc.scalar.activation(tneg[:], tneg[:], AF.Exp)
            nc.vector.tensor_scalar_max(h2[:], h2[:], 0.0)
            nc.vector.tensor_tensor(out=h2[:], in0=h2[:], in1=tneg[:], op=OP.add)
            nc.vector.tensor_scalar_add(h2[:], h2[:], -1.0)
            hg2 = pp.tile([P, nt * D2], F32, tag="hg2")
            CB = 16
            for c0 in range(0, nt, CB):
                c1 = min(c0 + CB, nt)
                nn = c1 - c0
                tmw = tp.tile([P, CB * OUT * HID], F32, tag="tmw")
                tmv = tmw[:, :nn * OUT * HID].rearrange(
                    "p (n o j) -> p n o j", o=OUT, j=HID)
                nc.vector.tensor_tensor(
                    out=tmv,
                    in0=h2[:, c0 * HID:c1 * HID].rearrange(
                        "p (n j) -> p n j", j=HID)[:, :, None, :].to_broadcast(
                        [P, nn, OUT, HID]),
                    in1=w2t_t[:, None, :].to_broadcast(
                        [P, nn, OUT * HID]).rearrange("p n (o j) -> p n o j", o=OUT),
                    op=OP.mult)
                nc.vector.tensor_reduce(
                    out=hg2[:, c0 * D2:c1 * D2].rearrange(
                        "p (n j) -> p n j", j=D2)[:, :, 0:OUT],
                    in_=tmv, axis=AX.X, op=OP.add)
            hg2v = hg2[:].rearrange("p (n j) -> p n j", j=D2)
            ad2_t = pp.tile([P, nt], F32, tag="ad2")
            for (vec_t, dest) in ((a2s_t, hg2v[:, :, OUT:OUT + 1]),
                                  (a2d_t, ad2_t[:, :, None])):
                t_c = tp.tile([P, nt * OUT], F32, tag="amul2")
                tv3 = t_c[:].rearrange("p (n j) -> p n j", j=OUT)
                nc.vector.tensor_tensor(
                    out=tv3, in0=hg2v[:, :, 0:OUT],
                    in1=vec_t[:, None, :].to_broadcast([P, nt, OUT]), op=OP.mult)
                nc.vector.tensor_reduce(out=dest, in_=tv3[:, :, None, :],
                                        axis=AX.X, op=OP.add)
            nc.vector.tensor_copy(out=hg2v[:, :, OUT + 1:OUT + 2],
                                  in_=ones_t[:, :, None])
            nc.sync.dma_start(out=cc2_d[:], in_=hg2[:])
            nc.gpsimd.collective_compute(
                "AllGather", OP.bypass, replica_groups=groups,
                ins=[cc2_d[:]], outs=[tab2_d[:]])

            U2 = pp.tile([P, nt * D2], F32, tag="U2")
            nc.vector.memset(U2[:], 0.0)
            edge_layer(tab2_d, D2, ad2_t, U2)

            z2_t = tp.tile([P, nt], F32, tag="z")
            nc.vector.tensor_scalar_add(
                z2_t[:, :, None],
                U2[:].rearrange("p (n j) -> p n j", j=D2)[:, :, D2 - 1:D2], 1e-16)
            rec2_t = tp.tile([P, nt], F32, tag="rec")
            nc.vector.reciprocal(rec2_t[:], z2_t[:])
            o_t = pp.tile([P, nt * OUT], F32, tag="out")
            nc.vector.tensor_tensor(
                out=o_t[:].rearrange("p (n j) -> p n j", j=OUT),
                in0=U2[:].rearrange("p (n j) -> p n j", j=D2)[:, :, 0:OUT],
                in1=rec2_t[:, :, None].to_broadcast([P, nt, OUT]), op=OP.mult)
            nc.sync.dma_start(out=out_d[:], in_=o_t[:])
    nc.finalize()
    return nc


def kernel(V, E, X, W1, a1_src, a1_dst, W2, a2_src, a2_dst):
    meta = _preprocess(E, X)
    nc = _build(meta)

    from concourse.bass_utils import run_bass_kernel_spmd

    in_maps = []
    for c in range(N_CORES):
        in_maps.append({
            "XcT": np.ascontiguousarray(meta.XcT[c]),
            "ones": np.ascontiguousarray(meta.ones[c]),
            "idx": np.ascontiguousarray(meta.idx[c]),
            "W1": np.asarray(W1, np.float32),
            "a1s": np.asarray(a1_src, np.float32).reshape(1, -1),
            "a1d": np.asarray(a1_dst, np.float32).reshape(1, -1),
            "W2T": np.ascontiguousarray(np.asarray(W2, np.float32).T).reshape(1, -1),
            "a2s": np.asarray(a2_src, np.float32).reshape(1, -1),
            "a2d": np.asarray(a2_dst, np.float32).reshape(1, -1),
        })
    res = run_bass_kernel_spmd(nc, in_maps, list(range(N_CORES)))

    out = np.zeros((N_NODES, OUT), dtype=np.float32)
    for c in range(N_CORES):
        g = res.results[c]["out"].reshape(P, meta.nrow_tot, OUT)
        pp, ii = np.nonzero(meta.cell2node[c] >= 0)
        ls = meta.cell2node[c, pp, ii]
        out[c * meta.NLOC + ls] = g[pp, ii]
    return out



# revision 3
# speedup vs baseline: 1.2669x; 1.2669x over previous
"""2-layer GAT on 8 Trainium2 NeuronCores — bulk-gather redesign.

Core c owns destination nodes [c*12500, (c+1)*12500); every edge lives on the
core that owns its destination. Edges are further split into 4 sub-grids by
source range (25000 nodes each) so that a table row index fits dma_gather's
int16 offset (one gather window per sub-grid). Each sub-grid is an
independent slot grid: its nodes are bucketed by that grid's local in-degree
(DP-optimized boundaries shared across cores, counts per-core), giving runs
of R contiguous slots per node along the free axis.

Per layer:
  1. Canonical table build: rows [H | alpha_src | alpha_dst | pad] at 256B
     stride; core's own block written locally (cc_d) and packed columns
     AllGathered + locally expanded into the full gather table (tab_d).
  2. Per sub-grid: one dma_gather per ~16K-slot chunk pulls per-edge rows,
     a small self-gather (from cc_d, per-grid cell order) provides
     alpha_dst per cell; DVE computes w=exp(leakyrelu(as+ad)) and reduces
     w*H and w per node run; dma_scatter_add accumulates the partial
     [sum wH | sum w] into a canonical per-node HBM buffer (unique idxs).
  3. Normalize U/(z+1e-16) in canonical cell layout, apply elu (layer 1),
     build next layer's table.

Pad slots gather a dummy row with alpha_src=-60 -> w ~ 6e-6, negligible
vs real z (~30 edges * O(1)); pad cells scatter into a dummy canonical row.
"""
import sys

sys.path.insert(0, "/opt/trn_rl_repo")

import numpy as np

P = 128
N_NODES = 100000
N_CORES = 8
NLOC = 12500           # dst nodes per core
NGRID = 4              # src-range sub-grids
SRCW = 25000           # src nodes per sub-grid window
IN_DIM = 256
HID = 8
OUT = 16
NEG = 0.2
NTC = (NLOC + P - 1) // P      # canonical cell columns = 98
TROW = 64                      # table row width (fp32) = 256B
BLK = NLOC + 1                 # per-core table block rows (+1 dummy)
DUMMY_AS = -60.0
IDX_CHUNK = 11264              # max idxs per dma_gather


class _Meta:
    pass


def _bucket_dp(cnts, dmax):
    """cnts: [C, dmax+1] per-core node counts by degree (deg>=1).
    Choose bucket boundaries minimizing sum_b nrow_b * R_b where
    nrow_b = max_c ceil(count_in_bucket/128), R_b = top degree of bucket."""
    pref = cnts.cumsum(axis=1)
    INF = 1 << 60
    fdp = [0] + [INF] * dmax
    chx = [0] * (dmax + 1)
    for j in range(1, dmax + 1):
        for i in range(1, j + 1):
            n = pref[:, j] - pref[:, i - 1]
            v = fdp[i - 1] + int(np.ceil(n.max() / P)) * j
            if v < fdp[j]:
                fdp[j] = v
                chx[j] = i
    deg2R = np.zeros(dmax + 1, dtype=np.int64)
    j = dmax
    while j > 0:
        i = chx[j]
        deg2R[i:j + 1] = j
        j = i - 1
    return deg2R


def _wrap_rep(idx):
    """[n] int -> [128, n/16] int16 wrapped p-major, tiled to 128 partitions."""
    n = len(idx)
    assert n % 16 == 0
    w = np.asarray(idx, np.int16).reshape(n // 16, 16).T
    return np.tile(w, (8, 1)).copy()


def _preprocess(E, X):
    src = np.asarray(E[0], dtype=np.int64)
    dst = np.asarray(E[1], dtype=np.int64)
    C, G = N_CORES, NGRID

    ecore = dst // NLOC
    eloc = dst % NLOC
    egrid = src // SRCW

    meta = _Meta()
    grids = []
    for g in range(G):
        m = _Meta()
        # local in-degree per (core, node) counting only src in window g
        deg = np.zeros((C, NLOC), dtype=np.int64)
        sel = egrid == g
        np.add.at(deg.reshape(-1), ecore[sel] * NLOC + eloc[sel], 1)
        dmax = int(deg.max())
        cnts = np.zeros((C, dmax + 1), dtype=np.int64)
        for c in range(C):
            cnts[c] = np.bincount(deg[c][deg[c] > 0], minlength=dmax + 1)
        deg2R = _bucket_dp(cnts, dmax)
        Rv = deg2R[deg]                      # [C, NLOC] slot run length
        Rs = sorted(set(int(r) for r in np.unique(deg2R[1:]) if r > 0))
        nrow = {}
        for R in Rs:
            cnt = ((Rv == R) & (deg > 0)).sum(axis=1)
            nrow[R] = int(np.ceil(cnt.max() / P))
        nt = sum(nrow.values())              # cell columns in this grid
        colbase, cb = {}, 0
        slotbase, sb = {}, 0
        for R in Rs:
            colbase[R] = cb
            cb += nrow[R]
            slotbase[R] = sb
            sb += nrow[R] * R
        T = sb                               # slot columns in this grid
        # per-column R and slot base
        col_R = np.zeros(nt, dtype=np.int64)
        col_sb = np.zeros(nt + 1, dtype=np.int64)
        for R in Rs:
            col_R[colbase[R]:colbase[R] + nrow[R]] = R
        acc = 0
        for i in range(nt):
            col_sb[i] = acc
            acc += col_R[i]
        col_sb[nt] = acc

        # cells: per core, nodes with deg>0 grouped by bucket R, id order
        cell2node = np.full((C, P, nt), -1, dtype=np.int64)
        for c in range(C):
            for R in Rs:
                ls = np.nonzero((Rv[c] == R) & (deg[c] > 0))[0]
                k = np.arange(len(ls))
                cell2node[c, k % P, colbase[R] + k // P] = ls

        # edge slot assignment + gather idx array [C, T*128] int16
        eidx = np.full((C, T * P), NLOC, dtype=np.int16)   # dummy idx
        selg = np.nonzero(sel)[0]
        g_src = src[selg]
        g_core = ecore[selg]
        g_loc = eloc[selg]
        order = np.lexsort((g_src, g_loc, g_core))
        g_src, g_core, g_loc = g_src[order], g_core[order], g_loc[order]
        # rank of node within its (core,bucket) group
        noderank = np.full((C, NLOC), -1, dtype=np.int64)
        for c in range(C):
            for R in Rs:
                ls = np.nonzero((Rv[c] == R) & (deg[c] > 0))[0]
                noderank[c, ls] = np.arange(len(ls))
        # position within node's run
        key = g_core * NLOC + g_loc
        grp_start = np.searchsorted(key, np.arange(C * NLOC))
        pos = np.arange(len(key)) - grp_start[key]
        e_R = Rv[g_core, g_loc]
        e_rank = noderank[g_core, g_loc]
        e_p = e_rank % P
        e_i = e_rank // P
        sb_arr = np.array([slotbase[int(r)] for r in e_R])
        e_t = sb_arr + e_i * e_R + pos
        idxval = (g_src + g_src // NLOC - (NLOC * 2 + 2) * g).astype(np.int16)
        eidx[g_core, e_t * P + e_p] = idxval

        # cell idx (self-gather + scatter): [C, nt*128] int16
        cidx = np.full((C, nt * P), NLOC, dtype=np.int16)   # dummy row
        for c in range(C):
            pp, ii = np.nonzero(cell2node[c] >= 0)
            cidx[c, ii * P + pp] = cell2node[c, pp, ii].astype(np.int16)

        # chunks: consecutive cell columns, <= IDX_CHUNK slots*128 idxs
        chunks = []
        i0 = 0
        while i0 < nt:
            i1 = i0
            while i1 < nt and (col_sb[i1 + 1] - col_sb[i0]) * P <= IDX_CHUNK:
                i1 += 1
            inters = []
            for R in Rs:
                ia = max(i0, colbase[R])
                ib = min(i1, colbase[R] + nrow[R])
                if ia < ib:
                    inters.append((R, ia, ib, int(col_sb[ia])))
            chunks.append((i0, i1, int(col_sb[i0]), int(col_sb[i1]), inters))
            i0 = i1
        m.nt, m.T, m.chunks = nt, T, chunks
        m.cell2node = cell2node
        m.eidx = eidx
        m.cidx = cidx
        grids.append(m)

    meta.grids = grids
    # canonical XcT per core: [IN_DIM, P*NTC], col j (=p + 128*i) = X[c*NLOC+j]
    NPAD = P * NTC
    XcT = np.zeros((C, IN_DIM, NPAD), dtype=np.float32)
    Xf = np.asarray(X, np.float32)
    for c in range(C):
        XcT[c][:, :NLOC] = Xf[c * NLOC:(c + 1) * NLOC].T
    meta.XcT = XcT
    return meta


def _build(meta, stage="full"):
    import concourse.bass as bass
    import concourse.bacc as bacc
    import concourse.mybir as mybir
    import concourse.tile as tile

    F32 = mybir.dt.float32
    I16 = mybir.dt.int16
    AX = mybir.AxisListType
    OP = mybir.AluOpType
    AF = mybir.ActivationFunctionType

    KCH = IN_DIM // P
    NTAB = N_CORES * BLK          # 100008 global table rows
    NTABP = ((NTAB + P - 1) // P) * P   # 100096, /128 for expand DMA
    G = NGRID

    nc = bacc.Bacc(num_swdge_queues=4)
    XcT_d = nc.declare_dram_parameter("XcT", [IN_DIM, P * NTC], F32,
                                      isOutput=False)
    W1_d = nc.declare_dram_parameter("W1", [IN_DIM, HID], F32, isOutput=False)
    a1s_d = nc.declare_dram_parameter("a1s", [1, HID], F32, isOutput=False)
    a1d_d = nc.declare_dram_parameter("a1d", [1, HID], F32, isOutput=False)
    W2T_d = nc.declare_dram_parameter("W2T", [1, OUT * HID], F32,
                                      isOutput=False)
    a2s_d = nc.declare_dram_parameter("a2s", [1, OUT], F32, isOutput=False)
    a2d_d = nc.declare_dram_parameter("a2d", [1, OUT], F32, isOutput=False)
    eidx_d = [nc.declare_dram_parameter(f"eidx{g}", [P, meta.grids[g].T * 8],
                                        I16, isOutput=False) for g in range(G)]
    cidx_d = [nc.declare_dram_parameter(f"cidx{g}", [P, meta.grids[g].nt * 8],
                                        I16, isOutput=False) for g in range(G)]
    out_d = nc.declare_dram_parameter("out", [P, NTC * OUT], F32, isOutput=True)

    cc1_d = nc.dram_tensor("cc1", [BLK, TROW], F32)          # own block L1
    cc1p_d = nc.dram_tensor("cc1p", [BLK, HID + 2], F32)     # packed for AG
    tab1p_d = nc.dram_tensor("tab1p", [NTABP, HID + 2], F32, addr_space="Shared")
    tab1_d = nc.dram_tensor("tab1", [NTABP, TROW], F32)
    cc2_d = nc.dram_tensor("cc2", [BLK, TROW], F32)
    cc2p_d = nc.dram_tensor("cc2p", [BLK, OUT + 2], F32)
    tab2p_d = nc.dram_tensor("tab2p", [NTABP, OUT + 2], F32, addr_space="Shared")
    tab2_d = nc.dram_tensor("tab2", [NTABP, TROW], F32)
    ucan_d = nc.dram_tensor("ucan", [P * NTC, TROW], F32)
    groups = [list(range(N_CORES))]

    with tile.TileContext(nc) as tc:
        with (
            tc.tile_pool(name="persist", bufs=1) as pp,
            tc.tile_pool(name="xs", bufs=2) as xp,
            tc.tile_pool(name="gp", bufs=2) as gp,
            tc.tile_pool(name="sf", bufs=2) as sfp,
            tc.tile_pool(name="ew", bufs=2) as ewp,
            tc.tile_pool(name="ix", bufs=3) as ixp,
            tc.tile_pool(name="tmp", bufs=1) as tp,
            tc.tile_pool(name="ps", bufs=8, space="PSUM") as psp,
        ):
            w1_t = pp.tile([P, KCH * HID], F32, tag="w1")
            for k in range(KCH):
                nc.sync.dma_start(out=w1_t[:, k * HID:(k + 1) * HID],
                                  in_=W1_d[k * P:(k + 1) * P, :])
            a1s_t = pp.tile([P, HID], F32, tag="a1s")
            nc.sync.dma_start(out=a1s_t[:],
                              in_=a1s_d[0:1, :].to_broadcast([P, HID]))
            a1d_t = pp.tile([P, HID], F32, tag="a1d")
            nc.sync.dma_start(out=a1d_t[:],
                              in_=a1d_d[0:1, :].to_broadcast([P, HID]))
            w2t_t = pp.tile([P, OUT * HID], F32, tag="w2t")
            nc.sync.dma_start(out=w2t_t[:],
                              in_=W2T_d[0:1, :].to_broadcast([P, OUT * HID]))
            a2s_t = pp.tile([P, OUT], F32, tag="a2s")
            nc.sync.dma_start(out=a2s_t[:],
                              in_=a2s_d[0:1, :].to_broadcast([P, OUT]))
            a2d_t = pp.tile([P, OUT], F32, tag="a2d")
            nc.sync.dma_start(out=a2d_t[:],
                              in_=a2d_d[0:1, :].to_broadcast([P, OUT]))

            # small zero tile: zeroes ucan cols 0:OUT+2 and dummy rows
            zt = pp.tile([P, NTC * (OUT + 2)], F32, tag="zt")
            nc.vector.memset(zt[:], 0.0)
            dmy = pp.tile([1, TROW], F32, tag="dmy")
            nc.vector.memset(dmy[:], 0.0)

            # canonical node table tile (both layers reuse)
            tb = pp.tile([P, NTC * TROW], F32, tag="tb")
            nc.vector.memset(tb[:], 0.0)
            tbv = tb[:].rearrange("p (n j) -> p n j", j=TROW)

            def zero_ucan(dcols):
                nc.sync.dma_start(
                    out=ucan_d[:, 0:dcols].rearrange(
                        "(i p) j -> p i j", p=P),
                    in_=zt[:].rearrange("p (n j) -> p n j", j=OUT + 2)
                    [:, :, 0:dcols])

            NF = NLOC // P          # full canonical columns
            NRem = NLOC - NF * P     # remainder rows in column NF

            def write_block(cc_d, ccp_d, pcols):
                # own table block rows j=p+128i from tb cells + dummy row
                nc.sync.dma_start(
                    out=cc_d[0:NF * P, :].rearrange("(i p) j -> p i j", p=P),
                    in_=tbv[:, 0:NF, :])
                nc.sync.dma_start(
                    out=ccp_d[0:NF * P, :].rearrange("(i p) j -> p i j", p=P),
                    in_=tbv[:, 0:NF, 0:pcols])
                if NRem:
                    nc.sync.dma_start(
                        out=cc_d[NF * P:NLOC, :],
                        in_=tbv[0:NRem, NF, :])
                    nc.sync.dma_start(
                        out=ccp_d[NF * P:NLOC, :],
                        in_=tbv[0:NRem, NF, 0:pcols])
                nc.sync.dma_start(out=cc_d[NLOC:NLOC + 1, :], in_=dmy[:])
                nc.sync.dma_start(out=ccp_d[NLOC:NLOC + 1, :],
                                  in_=dmy[:, 0:pcols])

            def allgather_expand(ccp_d, tabp_d, tab_d, pcols):
                nc.gpsimd.collective_compute(
                    "AllGather", OP.bypass, replica_groups=groups,
                    ins=[ccp_d[:]], outs=[tabp_d[0:NTAB, :]])
                # expand packed rows into 256B-stride gather table via SBUF
                CH = 112
                ncols_tot = NTABP // P
                for i0 in range(0, ncols_tot, CH):
                    i1 = min(i0 + CH, ncols_tot)
                    nn = i1 - i0
                    ex = xp.tile([P, CH * (OUT + 2)], F32, tag="ex")
                    exv = ex[:, :nn * pcols].rearrange(
                        "p (n j) -> p n j", j=pcols)
                    nc.sync.dma_start(
                        out=exv,
                        in_=tabp_d[i0 * P:i1 * P, :].rearrange(
                            "(i p) j -> p i j", p=P))
                    nc.sync.dma_start(
                        out=tab_d[i0 * P:i1 * P, 0:pcols].rearrange(
                            "(i p) j -> p i j", p=P),
                        in_=exv)
                    r0 = i1

            # ---------- layer 1 table: H = X @ W1 (canonical cells)
            XB = 8
            for b0 in range(0, NTC, XB):
                b1 = min(b0 + XB, NTC)
                nb = b1 - b0
                xt = xp.tile([P, KCH * XB * P], F32, tag="xt")
                for k in range(KCH):
                    nc.sync.dma_start(
                        out=xt[:, k * XB * P:k * XB * P + nb * P],
                        in_=XcT_d[k * P:(k + 1) * P, b0 * P:b1 * P])
                for b in range(b0, b1):
                    ps = psp.tile([P, HID], F32, tag="hps")
                    for k in range(KCH):
                        nc.tensor.matmul(
                            out=ps[:],
                            lhsT=xt[:, k * XB * P + (b - b0) * P:
                                    k * XB * P + (b - b0 + 1) * P],
                            rhs=w1_t[:, k * HID:(k + 1) * HID],
                            start=(k == 0), stop=(k == KCH - 1))
                    nc.scalar.copy(out=tb[:, b * TROW:b * TROW + HID],
                                   in_=ps[:])

            def add_alpha(D, s_t, d_t):
                hv = tbv[:, :, 0:D]
                for (vec, col) in ((s_t, D), (d_t, D + 1)):
                    t_a = tp.tile([P, NTC * OUT], F32, tag="amul")
                    tv = t_a[:, :NTC * D].rearrange("p (n j) -> p n j", j=D)
                    nc.vector.tensor_tensor(
                        out=tv, in0=hv,
                        in1=vec[:, None, :].to_broadcast([P, NTC, D]),
                        op=OP.mult)
                    nc.vector.tensor_reduce(
                        out=tbv[:, :, col:col + 1], in_=tv[:, :, None, :],
                        axis=AX.X, op=OP.add)

            add_alpha(HID, a1s_t, a1d_t)
            nc.vector.memset(dmy[:, HID:HID + 1], DUMMY_AS)
            write_block(cc1_d, cc1p_d, HID + 2)
            allgather_expand(cc1p_d, tab1p_d, tab1_d, HID + 2)
            zero_ucan(HID + 1)

            # ---------- edge phase
            def edge_phase(cc_d, tab_d, ucan_cols, D):
                if stage == "tab":
                    return
                AS_COL = D        # alpha_src column in table row
                AD_COL = D + 1
                for g in range(G):
                    m = meta.grids[g]
                    nt, T = m.nt, m.T
                    # self-gather: own rows in grid-cell order -> ad per cell
                    ad_t = pp.tile([P, 512], F32, tag=f"ad{g}")
                    cix = ixp.tile([P, nt * 8], I16, tag="cix")
                    nc.sync.dma_start(out=cix[:], in_=cidx_d[g][:])
                    SFC = 35
                    for s0 in range(0, nt, SFC):
                        s1 = min(s0 + SFC, nt)
                        ns = s1 - s0
                        sf = sfp.tile([P, SFC * TROW], F32, tag="sf")
                        nc.gpsimd.dma_gather(
                            sf[:, :ns * TROW].rearrange(
                                "p (c e) -> p c e", e=TROW),
                            cc_d[:],
                            cix[:, s0 * 8:s0 * 8 + ns * 8],
                            ns * P, ns * P, TROW, single_packet=False,
                            queue_num=g)
                        nc.scalar.copy(
                            out=ad_t[:, s0:s1, None],
                            in_=sf[:, :ns * TROW].rearrange(
                                "p (c e) -> p c e", e=TROW)[:, :, AD_COL:AD_COL + 1])
                    # U accumulator for this grid
                    ug = pp.tile([P, nt * (OUT + 1)], F32, tag=f"ug{g}")
                    nc.vector.memset(ug[:], 0.0)
                    ugv = ug[:, :nt * (D + 1)].rearrange(
                        "p (n j) -> p n j", j=D + 1)
                    if stage == "selfg":
                        continue
                    for (i0, i1, s0, s1, inters) in m.chunks:
                        SC = s1 - s0
                        eix = ixp.tile([P, IDX_CHUNK // 16], I16, tag="eix")
                        nc.sync.dma_start(out=eix[:, :SC * 8],
                                          in_=eidx_d[g][:, s0 * 8:s1 * 8])
                        gt = gp.tile([P, (IDX_CHUNK // P) * TROW], F32,
                                     tag="gt")
                        gv = gt[:, :SC * TROW].rearrange(
                            "p (c e) -> p c e", e=TROW)
                        nc.gpsimd.dma_gather(
                            gv, tab_d[(2 * NLOC + 2) * g:
                                      (2 * NLOC + 2) * g + 2 * NLOC + 2, :],
                            eix[:, :SC * 8], SC * P, SC * P, TROW,
                            single_packet=False, queue_num=g)
                        e_t = ewp.tile([P, IDX_CHUNK // P], F32, tag="e")
                        w_t = ewp.tile([P, IDX_CHUNK // P], F32, tag="w")
                        for (R, ia, ib, sa) in inters:
                            nn = ib - ia
                            o = sa - s0
                            ev = e_t[:, o:o + nn * R].rearrange(
                                "p (n r) -> p n r", r=R)
                            gsv = gt[:, o * TROW:(o + nn * R) * TROW].rearrange(
                                "p (n r j) -> p n r j", r=R, j=TROW)[:, :, :, AS_COL]
                            adv = ad_t[:, ia:ib, None].to_broadcast([P, nn, R])
                            nc.vector.tensor_tensor(out=ev, in0=gsv, in1=adv,
                                                    op=OP.add)
                        nc.vector.tensor_scalar_mul(w_t[:, :SC], e_t[:, :SC],
                                                    NEG)
                        nc.vector.tensor_tensor(out=w_t[:, :SC],
                                                in0=w_t[:, :SC],
                                                in1=e_t[:, :SC], op=OP.max)
                        nc.scalar.activation(w_t[:, :SC], w_t[:, :SC], AF.Exp)
                        nc.vector.tensor_tensor(
                            out=gv[:, :, 0:D], in0=gv[:, :, 0:D],
                            in1=w_t[:, :SC, None].to_broadcast([P, SC, D]),
                            op=OP.mult)
                        for (R, ia, ib, sa) in inters:
                            nn = ib - ia
                            o = sa - s0
                            gmv = gt[:, o * TROW:(o + nn * R) * TROW].rearrange(
                                "p (n r j) -> p n j r", r=R, j=TROW)[:, :, 0:D, :]
                            nc.vector.tensor_reduce(
                                out=ugv[:, ia:ib, 0:D], in_=gmv,
                                axis=AX.X, op=OP.add)
                            wv = w_t[:, o:o + nn * R].rearrange(
                                "p (n r) -> p n r", r=R)[:, :, None, :]
                            nc.vector.tensor_reduce(
                                out=ugv[:, ia:ib, D:D + 1], in_=wv,
                                axis=AX.X, op=OP.add)
                    if stage != "noscatter":
                        SCC = 28     # cell-cols per scatter: 2*28*128/16+1
                        for i0 in range(0, nt, SCC):
                            i1 = min(i0 + SCC, nt)
                            nn = i1 - i0
                            nc.gpsimd.dma_scatter_add(
                                ucan_d[:, 0:D + 1],
                                ugv[:, i0:i1, :],
                                cix[:, i0 * 8:i1 * 8], nn * P, nn * P, D + 1,
                                elem_step=TROW, single_packet=False,
                                queue_num=g)

            edge_phase(cc1_d, tab1_d, HID + 1, HID)

            # ---------- post layer 1: h2 = elu(U/z) -> layer-2 table
            uc = pp.tile([P, NTC * (OUT + 2)], F32, tag="uc")
            D1 = HID + 1
            ucv = uc[:, :NTC * D1].rearrange("p (n j) -> p n j", j=D1)
            nc.sync.dma_start(
                out=ucv,
                in_=ucan_d[0:P * NTC, 0:D1].rearrange(
                    "(i p) j -> p i j", p=P))
            z_t = tp.tile([P, NTC], F32, tag="z")
            nc.vector.tensor_scalar_add(z_t[:, :, None],
                                        ucv[:, :, HID:HID + 1], 1e-16)
            rec_t = tp.tile([P, NTC], F32, tag="rec")
            nc.vector.reciprocal(rec_t[:], z_t[:])
            h2 = pp.tile([P, NTC * HID], F32, tag="h2")
            h2v = h2[:].rearrange("p (n j) -> p n j", j=HID)
            nc.vector.tensor_tensor(
                out=h2v, in0=ucv[:, :, 0:HID],
                in1=rec_t[:, :, None].to_broadcast([P, NTC, HID]), op=OP.mult)
            tneg = tp.tile([P, NTC * HID], F32, tag="telu")
            nc.vector.tensor_scalar_min(tneg[:], h2[:], 0.0)
            nc.scalar.activation(tneg[:], tneg[:], AF.Exp)
            nc.vector.tensor_scalar_max(h2[:], h2[:], 0.0)
            nc.vector.tensor_tensor(out=h2[:], in0=h2[:], in1=tneg[:],
                                    op=OP.add)
            nc.vector.tensor_scalar_add(h2[:], h2[:], -1.0)

            # H2 = h2 @ W2 into tb cells
            nc.vector.memset(tb[:], 0.0)
            CB = 8
            for c0 in range(0, NTC, CB):
                c1 = min(c0 + CB, NTC)
                nn = c1 - c0
                tmw = tp.tile([P, CB * OUT * HID], F32, tag="tmw")
                tmv = tmw[:, :nn * OUT * HID].rearrange(
                    "p (n o j) -> p n o j", o=OUT, j=HID)
                nc.vector.tensor_tensor(
                    out=tmv,
                    in0=h2v[:, c0:c1, None, :].to_broadcast([P, nn, OUT, HID]),
                    in1=w2t_t[:, None, :].to_broadcast(
                        [P, nn, OUT * HID]).rearrange(
                        "p n (o j) -> p n o j", o=OUT),
                    op=OP.mult)
                nc.vector.tensor_reduce(
                    out=tbv[:, c0:c1, 0:OUT], in_=tmv, axis=AX.X, op=OP.add)
            add_alpha(OUT, a2s_t, a2d_t)
            nc.vector.memset(dmy[:, OUT:OUT + 1], DUMMY_AS)
            write_block(cc2_d, cc2p_d, OUT + 2)
            allgather_expand(cc2p_d, tab2p_d, tab2_d, OUT + 2)
            zero_ucan(OUT + 1)

            edge_phase(cc2_d, tab2_d, OUT + 1, OUT)

            # ---------- post layer 2: out = U/z
            D2 = OUT + 1
            uc2v = uc[:, :NTC * D2].rearrange("p (n j) -> p n j", j=D2)
            nc.sync.dma_start(
                out=uc2v,
                in_=ucan_d[0:P * NTC, 0:D2].rearrange(
                    "(i p) j -> p i j", p=P))
            nc.vector.tensor_scalar_add(z_t[:, :, None],
                                        uc2v[:, :, OUT:OUT + 1], 1e-16)
            nc.vector.reciprocal(rec_t[:], z_t[:])
            o_t = pp.tile([P, NTC * OUT], F32, tag="out")
            nc.vector.tensor_tensor(
                out=o_t[:].rearrange("p (n j) -> p n j", j=OUT),
                in0=uc2v[:, :, 0:OUT],
                in1=rec_t[:, :, None].to_broadcast([P, NTC, OUT]), op=OP.mult)
            nc.sync.dma_start(out=out_d[:], in_=o_t[:])
    nc.finalize()
    return nc


def _in_maps(meta, W1, a1_src, a1_dst, W2, a2_src, a2_dst):
    maps = []
    for c in range(N_CORES):
        m = {
            "XcT": np.ascontiguousarray(meta.XcT[c]),
            "W1": np.asarray(W1, np.float32),
            "a1s": np.asarray(a1_src, np.float32).reshape(1, -1),
            "a1d": np.asarray(a1_dst, np.float32).reshape(1, -1),
            "W2T": np.ascontiguousarray(
                np.asarray(W2, np.float32).T).reshape(1, -1),
            "a2s": np.asarray(a2_src, np.float32).reshape(1, -1),
            "a2d": np.asarray(a2_dst, np.float32).reshape(1, -1),
        }
        for g in range(NGRID):
            m[f"eidx{g}"] = _wrap_rep(meta.grids[g].eidx[c])
            m[f"cidx{g}"] = _wrap_rep(meta.grids[g].cidx[c])
        maps.append(m)
    return maps


def _unshard(meta, results):
    out = np.zeros((N_NODES, OUT), dtype=np.float32)
    for c in range(N_CORES):
        g = results[c]["out"].reshape(P, NTC, OUT)
        j = np.arange(NLOC)
        out[c * NLOC + j] = g[j % P, j // P]
    return out


def kernel(V, E, X, W1, a1_src, a1_dst, W2, a2_src, a2_dst):
    meta = _preprocess(E, X)
    nc = _build(meta)
    from concourse.bass_utils import run_bass_kernel_spmd
    maps = _in_maps(meta, W1, a1_src, a1_dst, W2, a2_src, a2_dst)
    res = run_bass_kernel_spmd(nc, maps, list(range(N_CORES)))
    return _unshard(meta, res.results)
